# revision 5
# baseline (speedup 1.0000x reference)
"""Trainium2 Bass kernel for nn_Attention_19739669692939 (sparse_attention).

Reference computation (shapes: L=1024, B=64, C=1024, D=512, E=512):
    Wa_e = W_attn[:, :C]        # [E, C]
    Wa_s = W_attn[:, C:]        # [E, D]
    pre  = enc_output @ Wa_e.T + s @ Wa_s.T     # [L, B, E] (s broadcast over L)
    engry = tanh(pre)
    att[b, l] = engry[l, b, :] @ W_v[0, :]
    out = softmax(att, axis=-1)                 # [B, 1024]

Distribution: pure data-parallel over batch. Core i handles batches
[8i, 8i+8); no collectives. Each core does a 8192x1024 @ 1024x512 matmul
(float32r on the PE: 1 cycle/row, ~1.5e-4 rel err), fused bias+tanh on ACT,
and a masked-weight matmul to reduce against W_v directly into per-batch
PSUM rows, then a free-axis softmax.

enc must be presented to the PE with the contraction dim (c) on partitions,
so each [128 l, 128 c] tile is transposed on the PE (is_transpose matmul
against identity, f32r = 1.5 cycles/row) and copied PSUM->SBUF on DVE/ACT.
All matmul-path tensors are declared float32r; the PE rounds internally, so
plain DMA of the f32 bits is sufficient (verified on HW).
"""

import numpy as np

import concourse.bass as bass
import concourse.mybir as mybir
from concourse import bacc
from concourse.bass_utils import run_bass_kernel_spmd
from concourse.masks import make_identity
from concourse.tile import TileContext

F32 = mybir.dt.float32
F32R = mybir.dt.float32r
AF = mybir.ActivationFunctionType

L = 1024          # enc length
B = 64            # global batch
BL = 8            # batch per core
C = 1024          # enc feature dim (2*enc_hid)
D = 512           # dec feature dim
E = 512           # engry dim
NCORES = 8

NCB = C // 128    # 8 c-blocks
NDB = D // 128    # 4 d-blocks
NEB = E // 128    # 4 e-blocks
LCH = 512         # l-chunk processed per inner iteration
NLC = L // LCH    # 2 chunks
KSUB = LCH // 128  # 4 l-subblocks per chunk


def build_nc():
    nc = bacc.Bacc("TRN2", target_bir_lowering=False, debug=False)

    enc = nc.dram_tensor("enc_output", [L, BL, C], F32R, kind="ExternalInput").ap()
    s = nc.dram_tensor("s", [1, BL, D], F32R, kind="ExternalInput").ap()
    w_attn = nc.dram_tensor("W_attn", [E, C + D], F32R, kind="ExternalInput").ap()
    w_v = nc.dram_tensor("W_v", [1, E], F32R, kind="ExternalInput").ap()
    out = nc.dram_tensor("out", [BL, L], F32, kind="ExternalOutput").ap()

    NWB = (C + D) // 128  # 12 blocks over W_attn's column (c/d) axis

    with TileContext(nc) as tc:
        with (
            tc.tile_pool(name="consts", bufs=1) as consts,
            tc.tile_pool(name="nat", bufs=2) as nat_pool,
            tc.tile_pool(name="encT", bufs=2) as encT_pool,
            tc.tile_pool(name="engry", bufs=2) as engry_pool,
            tc.tile_pool(name="tp", bufs=2, space="PSUM") as tp_pool,
            tc.tile_pool(name="pre", bufs=2, space="PSUM") as pre_pool,
            tc.tile_pool(name="att", bufs=2, space="PSUM") as att_pool,
        ):
            # ---------------- setup: constants and weights ----------------
            ident = consts.tile([128, 128], F32, tag="ident")
            make_identity(nc, ident)
            identR = consts.tile([128, 128], F32R, tag="identR")
            nc.vector.tensor_copy(identR[:], ident[:])

            # W_attn natural: [e(4x128 part), c+d(1536)]
            wnat = consts.tile([128, NEB * (C + D)], F32R, tag="wnat")
            for r in range(NEB):
                nc.sync.dma_start(
                    out=wnat[:, r * (C + D):(r + 1) * (C + D)],
                    in_=w_attn[r * 128:(r + 1) * 128, :],
                )
            # WaT: [cd(128 part), (cd_block, e)] — W_attn.T in 12 blocks of
            # [128, 512].
            waT = consts.tile([128, NWB * E], F32R, tag="waT")
            for ci in range(NWB):
                tpw = tp_pool.tile([128, 512], F32R, tag="tp")
                for r in range(NEB):
                    nc.tensor.transpose(
                        tpw[:, r * 128:(r + 1) * 128],
                        wnat[:, r * (C + D) + ci * 128: r * (C + D) + (ci + 1) * 128],
                        identR[:],
                    )
                if ci % 2 == 0:
                    nc.vector.tensor_copy(waT[:, ci * E:(ci + 1) * E], tpw[:])
                else:
                    nc.scalar.copy(waT[:, ci * E:(ci + 1) * E], tpw[:])

            # s: [1, BL, D] -> s_sbuf [BL, D] -> sT [d(4x128 part), b(8)]
            s_sbuf = consts.tile([BL, D], F32R, tag="s_sbuf")
            nc.sync.dma_start(out=s_sbuf[:], in_=s[0])
            sT = consts.tile([128, NDB * BL], F32R, tag="sT")
            for db in range(NDB):
                tps = tp_pool.tile([128, 512], F32R, tag="tp")
                nc.tensor.transpose(
                    tps[:, :BL],
                    s_sbuf[:, db * 128:(db + 1) * 128],
                    identR[:BL, :BL],
                )
                nc.vector.tensor_copy(sT[:, db * BL:(db + 1) * BL], tps[:, :BL])

            # bias[e, b] = Wa_s @ s[b].T  — [e(4x128 part), b(8)] per e-block
            bias_sbuf = consts.tile([128, NEB * BL], F32, tag="bias")
            for eb in range(NEB):
                bps = tp_pool.tile([128, 512], F32, tag="tp")
                for db in range(NDB):
                    nc.tensor.matmul(
                        bps[:, :BL],
                        lhsT=waT[:, (NCB + db) * E + eb * 128:
                                 (NCB + db) * E + (eb + 1) * 128],
                        rhs=sT[:, db * BL:(db + 1) * BL],
                        start=(db == 0),
                        stop=(db == NDB - 1),
                    )
                nc.vector.tensor_copy(bias_sbuf[:, eb * BL:(eb + 1) * BL], bps[:, :BL])

            # W_v: [1, E] -> wvT [e(128 part), eb(4)] — f32 path (fp32r
            # matmuls need even free counts; N=1 transpose is illegal).
            wv_sbuf = consts.tile([1, E], F32, tag="wv_sbuf")
            nc.sync.dma_start(out=wv_sbuf[:], in_=w_v.bitcast(F32)[:])
            wvT = consts.tile([128, NEB], F32, tag="wvT")
            for eb in range(NEB):
                tpv = tp_pool.tile([128, 512], F32, tag="tp")
                nc.tensor.transpose(
                    tpv[:, :1],
                    wv_sbuf[:, eb * 128:(eb + 1) * 128],
                    ident[:1, :1],
                )
                nc.vector.tensor_copy(wvT[:, eb:eb + 1], tpv[:, :1])

            # Masked W_v weights: for each (eb, b) a [128, 8] tile whose
            # column b holds wvT[:, eb], zeros elsewhere. Lets the W_v
            # contraction land in PSUM row b for batch b. Built in f32,
            # cast-copied once to f32r for the PE.
            wv_maskF = consts.tile([128, NEB * BL * BL], F32, tag="wv_maskF")
            nc.vector.memset(wv_maskF[:], 0.0)
            for eb in range(NEB):
                for b in range(BL):
                    nc.vector.tensor_copy(
                        wv_maskF[:, eb * BL * BL + b * BL + b:
                                 eb * BL * BL + b * BL + b + 1],
                        wvT[:, eb:eb + 1],
                    )
            wv_mask = consts.tile([128, NEB * BL * BL], F32R, tag="wv_mask")
            nc.vector.tensor_copy(wv_mask[:], wv_maskF[:])

            att_sbuf = consts.tile([BL, L], F32, tag="att_sbuf")

            # ---------------- main loop ----------------
            for lc in range(NLC):
                att_ps = att_pool.tile([BL, LCH], F32, tag="att")
                for b in range(BL):
                    l0 = lc * LCH
                    enc_t = nat_pool.tile([128, KSUB * C], F32R, tag="nat")
                    nc.sync.dma_start(
                        out=enc_t.rearrange("p (k c) -> p k c", k=KSUB),
                        in_=enc[l0:l0 + LCH, b, :].rearrange(
                            "(k p) c -> p k c", p=128
                        ),
                    )

                    encT = [
                        encT_pool.tile([128, LCH], F32R, tag=f"encT{cb}",
                                       name=f"encT{cb}_{lc}_{b}")
                        for cb in range(NCB)
                    ]
                    for cb in range(NCB):
                        tpt = tp_pool.tile([128, 512], F32R, tag="tp")
                        for k in range(KSUB):
                            nc.tensor.transpose(
                                tpt[:, k * 128:(k + 1) * 128],
                                enc_t[:, k * C + cb * 128: k * C + (cb + 1) * 128],
                                identR[:],
                            )
                        if cb < 5:
                            nc.vector.tensor_copy(encT[cb][:], tpt[:])
                        else:
                            nc.scalar.copy(encT[cb][:], tpt[:])

                    for eb in range(NEB):
                        pre = pre_pool.tile([128, LCH], F32, tag="pre")
                        for cb in range(NCB):
                            nc.tensor.matmul(
                                pre[:],
                                lhsT=waT[:, cb * E + eb * 128:
                                         cb * E + (eb + 1) * 128],
                                rhs=encT[cb][:],
                                start=(cb == 0),
                                stop=(cb == NCB - 1),
                            )
                        engry = engry_pool.tile([128, LCH], F32R, tag=f"engry{eb}")
                        nc.scalar.activation(
                            engry[:], pre[:], AF.Tanh,
                            bias=bias_sbuf[:, eb * BL + b: eb * BL + b + 1],
                            scale=1.0,
                        )
                        nc.tensor.matmul(
                            att_ps[:],
                            lhsT=wv_mask[:, eb * BL * BL + b * BL:
                                         eb * BL * BL + (b + 1) * BL],
                            rhs=engry[:],
                            start=(b == 0 and eb == 0),
                            stop=(b == BL - 1 and eb == NEB - 1),
                        )
                nc.vector.tensor_copy(att_sbuf[:, lc * LCH:(lc + 1) * LCH], att_ps[:])

            # ---------------- softmax over l ----------------
            mx = consts.tile([BL, 1], F32, tag="mx")
            nmx = consts.tile([BL, 1], F32, tag="nmx")
            att_e = consts.tile([BL, L], F32, tag="att_e")
            sm = consts.tile([BL, 1], F32, tag="sm")
            rs = consts.tile([BL, 1], F32, tag="rs")
            att_o = consts.tile([BL, L], F32, tag="att_o")

            nc.vector.reduce_max(mx[:], att_sbuf[:], axis=mybir.AxisListType.X)
            nc.scalar.mul(nmx[:], mx[:], -1.0)
            nc.scalar.activation(att_e[:], att_sbuf[:], AF.Exp,
                                 bias=nmx[:, 0:1], scale=1.0)
            nc.vector.reduce_sum(sm[:], att_e[:], axis=mybir.AxisListType.X)
            nc.vector.reciprocal(rs[:], sm[:])
            nc.vector.tensor_scalar_mul(att_o[:], att_e[:], rs[:, 0:1])
            nc.sync.dma_start(out=out[:], in_=att_o[:])

    nc.compile()
    return nc


_NC_CACHE = None


def _get_nc():
    global _NC_CACHE
    if _NC_CACHE is None:
        _NC_CACHE = build_nc()
    return _NC_CACHE


def make_in_maps(enc_output, s, W_attn, W_v):
    enc_output = np.asarray(enc_output, dtype=np.float32)
    s = np.asarray(s, dtype=np.float32)
    W_attn = np.ascontiguousarray(np.asarray(W_attn, dtype=np.float32))
    W_v = np.ascontiguousarray(np.asarray(W_v, dtype=np.float32))
    in_maps = []
    for i in range(NCORES):
        in_maps.append({
            "enc_output": np.ascontiguousarray(enc_output[:, i * BL:(i + 1) * BL, :]),
            "s": np.ascontiguousarray(s[:, i * BL:(i + 1) * BL, :]),
            "W_attn": W_attn,
            "W_v": W_v,
        })
    return in_maps


def kernel(enc_output, s, W_attn, W_v):
    nc = _get_nc()
    in_maps = make_in_maps(enc_output, s, W_attn, W_v)
    res = run_bass_kernel_spmd(nc, in_maps, core_ids=list(range(NCORES)))
    return np.concatenate([res.results[i]["out"] for i in range(NCORES)], axis=0)
